# revision 14
# baseline (speedup 1.0000x reference)
"""Trainium2 Bass kernel for ContentSelectionCell (fp16 datapath).

Computes, for full inputs x[64,512], enc_outs[64,2048,512], W[1024,512], b[512],
actual_step scalar:

    scores  = einsum('bd,btd->bt', x, enc_outs); scores[:, step] = -1e9
    align   = softmax(scores, -1)
    context = einsum('bt,btd->bd', align, enc_outs)
    att     = sigmoid(concat([x, context], -1) @ W + b)
    out     = att * x

Sharding: data-parallel over batch, 8 batches per core on 8 NeuronCores.

Perf design (vs fp32 baseline at ~151us):
  - enc is uploaded as fp16 from host: HBM traffic halves to 16 MB/core
    (~45us DMA floor at 358 GB/s).  fp16 (not bf16) keeps softmax-logit
    noise ~2^-11*sqrt(512) ~ 0.01, giving ~1.6e-3 final rel err.
  - scores einsum is split across two engines, since every fused DVE
    reduce op (TTR/STT) runs at 1x mode only (~594ns per [128,512] chunk):
      * NPROD chunks: one DVE tensor_tensor multiply at 2x_1p fp16 mode
        producing fp16 products, reduced per-chunk on the otherwise-idle
        Activation engine via Copy+accum_out.
      * remaining chunks: fused DVE tensor_tensor_reduce at 1x.
  - softmax uses the exact per-row max (cheap cross-partition max via PE
    transpose + ones-matmul broadcast); required so fp16 expv can't
    over/underflow.
  - context accumulated on PE: 16 matmuls [K=128,M=1,N=512] fp16/batch.
  - final Dense over all 8 local rows with host-pretransposed x^T/W-chunk
    fp16 layouts; ctx^T columns built per batch via tiny PE transposes
    (single-partition writes must start at partition 0, so a stacked
    [8,512] staging tile + DVE block transpose is not expressible).
"""

import os
from contextlib import ExitStack

import numpy as np

import concourse.bacc as bacc
import concourse.bass as bass
import concourse.tile as tile
from concourse import mybir
from concourse.bass_utils import run_bass_kernel_spmd
from concourse.masks import make_identity

N_CORES = 8
B, T, D, H = 64, 2048, 512, 512
BL = B // N_CORES  # local batches per core
KCH = 16           # T chunks per batch: t = 16*p + k
NEG = -1e9

F32 = mybir.dt.float32
F16 = mybir.dt.float16
AO = mybir.AluOpType
AF = mybir.ActivationFunctionType

# chunks reduced via DVE-product + ACT Copy-accum (must be even); the
# remaining KCH-NPROD chunks use the fused 1x DVE tensor_tensor_reduce.
NPROD = int(os.environ.get("CSEL_NPROD", "12"))
ENC_BUFS = int(os.environ.get("CSEL_ENC_BUFS", "5"))
# 1 = per-pair product ops with real APs instead of one op with a
# stride-0 mid-dim broadcast on in1 (HW-compat bisect knob)
NO_BCAST = bool(int(os.environ.get("CSEL_NO_BCAST", "0")))
# 1 = use scalar_tensor_tensor (baseline-proven) instead of
# tensor_tensor_reduce for the fused chunks
STT = bool(int(os.environ.get("CSEL_STT", "0")))

_CACHE = {}


def _ensure_ntff_hook():
    """Register the axon NTFF profiling hook if the image's antenv lacks it."""
    import sys
    import types

    try:
        from antenv.axon_hooks import get_axon_ntff_profile_hook  # noqa: F401

        return
    except ImportError:
        pass
    try:
        import antenv
        from trn_agent_boot.trn_boot import _ntff_profile_via_ctypes

        hook = _ntff_profile_via_ctypes("/opt/axon/libaxon_pjrt.so")
        mod = types.ModuleType("antenv.axon_hooks")
        mod._hook = hook
        mod.set_axon_ntff_profile_hook = lambda h: setattr(mod, "_hook", h)
        mod.get_axon_ntff_profile_hook = lambda: mod._hook
        sys.modules["antenv.axon_hooks"] = mod
        antenv.axon_hooks = mod

        import concourse.bass_utils as _bu

        _bu.upload_artifacts = lambda tmpdir: tmpdir
    except Exception:
        pass


def _build(nprod: int, no_bcast: bool = NO_BCAST, stt: bool = STT) -> bass.Bass:
    assert nprod % 2 == 0 and 0 <= nprod <= KCH
    nttr = KCH - nprod
    nc = bacc.Bacc(None)

    enc = nc.declare_dram_parameter("enc", [BL, T, D], F16, isOutput=False)
    xsf = nc.declare_dram_parameter("xsf", [1, BL * D], F16, isOutput=False)
    mask = nc.declare_dram_parameter("mask", [128, KCH], F32, isOutput=False)
    wT = nc.declare_dram_parameter("wT", [128, 8, H], F16, isOutput=False)
    xT = nc.declare_dram_parameter("xT", [128, 4, BL], F16, isOutput=False)
    bias = nc.declare_dram_parameter("bias", [1, H], F32, isOutput=False)
    xs = nc.declare_dram_parameter("xs", [BL, D], F32, isOutput=False)
    out = nc.declare_dram_parameter("out", [BL, D], F32, isOutput=True)

    with tile.TileContext(nc) as tc, ExitStack() as ctx:
        const = ctx.enter_context(tc.tile_pool(name="const", bufs=1))
        encp = ctx.enter_context(tc.tile_pool(name="encp", bufs=ENC_BUFS))
        prodp = ctx.enter_context(tc.tile_pool(name="prodp", bufs=2))
        work = ctx.enter_context(tc.tile_pool(name="work", bufs=3))
        ps_x = ctx.enter_context(tc.tile_pool(name="ps_x", bufs=2, space="PSUM"))
        ps_sm = ctx.enter_context(tc.tile_pool(name="ps_sm", bufs=3, space="PSUM"))
        ps_ctx = ctx.enter_context(tc.tile_pool(name="ps_ctx", bufs=2, space="PSUM"))
        ps_att = ctx.enter_context(tc.tile_pool(name="ps_att", bufs=1, space="PSUM"))

        # ---- constants ----
        ones_row16 = const.tile([1, 128], F16)
        nc.vector.memset(ones_row16, 1.0)
        ones_row32 = const.tile([1, 128], F32)
        nc.vector.memset(ones_row32, 1.0)
        ones_col = const.tile([128, 1], F32)
        nc.vector.memset(ones_col, 1.0)
        ones_b = const.tile([1, BL], F32)
        nc.vector.memset(ones_b, 1.0)
        id128 = const.tile([128, 128], F32)
        make_identity(nc, id128)
        id1 = const.tile([1, 1], F32)
        nc.vector.memset(id1, 1.0)

        # early consts (needed before/inside the batch loop)
        xsf_sb = const.tile([1, BL * D], F16)
        nc.sync.dma_start(xsf_sb, xsf[:])
        mask_sb = const.tile([128, KCH], F32)
        nc.sync.dma_start(mask_sb, mask[:])

        # x replicated to all partitions, twice along free dim (so FD=1024
        # product ops can use it without mid-dim broadcast): [128, b, 1024]
        xrep = const.tile([128, BL, 1, 2 * D], F16)
        for b in range(BL):
            xr_ps = ps_x.tile([128, D], F32, tag="xr", name=f"xr_{b}")
            nc.tensor.matmul(xr_ps, lhsT=ones_row16, rhs=xsf_sb[:, b * D : (b + 1) * D])
            nc.scalar.copy(xrep[:, b, 0, 0:D], xr_ps)
            nc.scalar.copy(xrep[:, b, 0, D : 2 * D], xr_ps)

        # ctx^T columns for the final dense, filled one batch at a time
        ctxT_sb = const.tile([128, 4, BL], F16)

        # ---- batch loop ----
        for b in range(BL):
            src = enc[b].rearrange("(p k) d -> p k d", p=128)
            eh = encp.tile([128, KCH, D], F16, tag="enc", name=f"enc_{b}")
            nc.sync.dma_start(eh, src)

            scores = work.tile([128, KCH], F32, tag="scores", name=f"scores_{b}")

            # product path: one 2x-mode multiply, per-chunk ACT reduce
            if nprod:
                prod = prodp.tile([128, nprod // 2, 2 * D], F16, tag="prod",
                                  name=f"prod_{b}")
                dummy_a = work.tile([128, D], F16, tag="dummy_a", name=f"dummy_a_{b}")
                if no_bcast:
                    for j in range(nprod // 2):
                        nc.vector.tensor_mul(
                            prod[:, j, :],
                            eh[:, 2 * j : 2 * j + 2, :].rearrange("p k d -> p (k d)"),
                            xrep[:, b, 0, :],
                        )
                else:
                    xr_b = xrep[:, b, :, :].broadcast_to((128, nprod // 2, 2 * D))
                    eh_flat = eh[:, 0:nprod, :].rearrange("p k d -> p (k d)")
                    nc.vector.tensor_mul(
                        prod.rearrange("p j d -> p (j d)"), eh_flat, xr_b
                    )
                pview = prod.rearrange("p j (two d) -> p (j two) d", two=2)
                for k in range(nprod):
                    nc.scalar.activation(
                        out=dummy_a,
                        in_=pview[:, k, :],
                        func=AF.Copy,
                        accum_out=scores[:, k : k + 1],
                    )
            # fused TTR path (1x) for the remaining chunks
            if nttr:
                dummy_t = work.tile([128, D], F16, tag="dummy_t", name=f"dummy_t_{b}")
                for k in range(nprod, KCH):
                    if stt:
                        nc.vector.scalar_tensor_tensor(
                            out=dummy_t,
                            in0=eh[:, k, :],
                            scalar=1.0,
                            in1=xrep[:, b, 0, 0:D],
                            op0=AO.mult,
                            op1=AO.mult,
                            accum_out=scores[:, k : k + 1],
                        )
                    else:
                        nc.vector.tensor_tensor_reduce(
                            out=dummy_t,
                            in0=eh[:, k, :],
                            in1=xrep[:, b, 0, 0:D],
                            scale=1.0,
                            scalar=0.0,
                            op0=AO.mult,
                            op1=AO.add,
                            accum_out=scores[:, k : k + 1],
                        )

            nc.vector.tensor_add(scores, scores, mask_sb)

            # exact per-row max -> -max broadcast to all partitions
            m1 = work.tile([128, 1], F32, tag="m1", name=f"m1_{b}")
            nc.vector.tensor_reduce(
                out=m1, in_=scores, axis=mybir.AxisListType.X, op=AO.max
            )
            mT_ps = ps_sm.tile([1, 128], F32, tag="small", name=f"mT_{b}")
            nc.tensor.transpose(mT_ps, m1, id128)
            mneg = work.tile([1, 1], F32, tag="mneg", name=f"mneg_{b}")
            nc.vector.tensor_reduce(
                out=mneg, in_=mT_ps, axis=mybir.AxisListType.X, op=AO.max, negate=True
            )
            negm_ps = ps_sm.tile([128, 1], F32, tag="small", name=f"negm_{b}")
            nc.tensor.matmul(negm_ps, lhsT=ones_row32, rhs=mneg)
            negm_sb = work.tile([128, 1], F32, tag="negm_sb", name=f"negm_sb_{b}")
            nc.scalar.copy(negm_sb, negm_ps)

            # exp (fp16 out; safe after exact-max shift) + per-partition sums
            expv = work.tile([128, KCH], F16, tag="expv", name=f"expv_{b}")
            s1 = work.tile([128, 1], F32, tag="s1", name=f"s1_{b}")
            nc.scalar.activation(
                out=expv, in_=scores, func=AF.Exp, bias=negm_sb, scale=1.0,
                accum_out=s1,
            )

            # unnormalized context on PE
            ctx_ps = ps_ctx.tile([1, D], F32, tag="ctx", name=f"ctx_{b}")
            for k in range(KCH):
                nc.tensor.matmul(
                    ctx_ps,
                    lhsT=expv[:, k : k + 1],
                    rhs=eh[:, k, :],
                    start=(k == 0),
                    stop=(k == KCH - 1),
                )

            # denominator: total = ones^T @ s1, then 1/total
            stot_ps = ps_sm.tile([1, 1], F32, tag="small", name=f"stot_{b}")
            nc.tensor.matmul(stot_ps, lhsT=ones_col, rhs=s1)
            rs = work.tile([1, 1], F32, tag="rs", name=f"rs_{b}")
            nc.vector.reciprocal(rs, stot_ps)

            # normalize on PSUM->SBUF copy
            ctxn = work.tile([1, D], F32, tag="ctxn", name=f"ctxn_{b}")
            nc.scalar.activation(
                out=ctxn, in_=ctx_ps, func=AF.Copy, bias=0.0, scale=rs[0:1, :]
            )
            # transpose [1, 512] -> 4 x [128, 1] columns for the dense lhsT
            ctxT_ps = ps_sm.tile([128, 4], F32, tag="small", name=f"ctxT_ps_{b}")
            for c in range(4):
                nc.tensor.transpose(
                    ctxT_ps[:, c : c + 1], ctxn[:, c * 128 : (c + 1) * 128], id1
                )
            nc.scalar.copy(ctxT_sb[:, :, b], ctxT_ps)

        # ---- tail: dense + gate ----
        # late consts (issued after all enc DMAs so they don't delay them)
        wT_sb = const.tile([128, 8, H], F16)
        nc.sync.dma_start(wT_sb, wT[:])
        xT_sb = const.tile([128, 4, BL], F16)
        nc.sync.dma_start(xT_sb, xT[:])
        bias_sb = const.tile([1, H], F32)
        nc.sync.dma_start(bias_sb, bias[:])
        xs_sb = const.tile([BL, D], F32)
        nc.sync.dma_start(xs_sb, xs[:])

        att_ps = ps_att.tile([BL, H], F32)
        nc.tensor.matmul(att_ps, lhsT=ones_b, rhs=bias_sb, start=True, stop=False)
        for c in range(4):
            nc.tensor.matmul(
                att_ps, lhsT=xT_sb[:, c, :], rhs=wT_sb[:, c, :],
                start=False, stop=False,
            )
        for c in range(4):
            nc.tensor.matmul(
                att_ps,
                lhsT=ctxT_sb[:, c, :],
                rhs=wT_sb[:, 4 + c, :],
                start=False,
                stop=(c == 3),
            )

        att_sb = work.tile([BL, H], F32, tag="att")
        nc.scalar.activation(att_sb, att_ps, AF.Sigmoid)
        res = work.tile([BL, D], F32, tag="res")
        nc.vector.tensor_mul(res, att_sb, xs_sb)
        nc.sync.dma_start(out[:], res)

    nc.finalize()
    return nc


def _get_nc() -> bass.Bass:
    key = (NPROD, NO_BCAST, STT)
    if key not in _CACHE:
        _CACHE[key] = _build(NPROD)
    return _CACHE[key]


LAST_RESULTS = None  # BassKernelResults of the most recent run (for test harness)


def kernel(x, enc_outs, W, b, actual_step, trace: bool = False) -> np.ndarray:
    x = np.ascontiguousarray(np.asarray(x, dtype=np.float32))
    enc = np.asarray(enc_outs, dtype=np.float32)
    W = np.asarray(W, dtype=np.float32)
    bvec = np.ascontiguousarray(np.asarray(b, dtype=np.float32)).reshape(1, H)
    step = int(np.asarray(actual_step))

    maskv = np.zeros(T, dtype=np.float32)
    if 0 <= step < T:
        maskv[step] = NEG
    mask2d = np.ascontiguousarray(maskv.reshape(128, KCH))

    enc16 = enc.astype(np.float16)
    x16 = x.astype(np.float16)
    W16 = W.astype(np.float16)
    # W in 8 row-chunks of 128: chunks 0..3 = x part, 4..7 = ctx part
    wT_h = np.ascontiguousarray(W16.reshape(8, 128, H).transpose(1, 0, 2))

    in_maps = []
    for i in range(N_CORES):
        xs_i = x[i * BL : (i + 1) * BL]
        x16_i = x16[i * BL : (i + 1) * BL]
        xT_i = np.ascontiguousarray(
            x16_i.T.reshape(4, 128, BL).transpose(1, 0, 2)
        )
        in_maps.append(
            {
                "enc": np.ascontiguousarray(enc16[i * BL : (i + 1) * BL]),
                "xsf": np.ascontiguousarray(x16_i.reshape(1, BL * D)),
                "mask": mask2d,
                "wT": wT_h,
                "xT": xT_i,
                "bias": bvec,
                "xs": np.ascontiguousarray(xs_i),
            }
        )

    nc = _get_nc()
    if trace:
        _ensure_ntff_hook()
    res = run_bass_kernel_spmd(nc, in_maps, core_ids=list(range(N_CORES)), trace=trace)
    global LAST_RESULTS
    LAST_RESULTS = res
    return np.concatenate([res.results[i]["out"] for i in range(N_CORES)], axis=0)
